# revision 1
# baseline (speedup 1.0000x reference)
"""Trainium2 Bass kernel for the 2-layer GCN (nn_CGNN_70566312673786).

Strategy (8 NeuronCores, SPMD):
  - Nodes (rows of x / segment_sum outputs) are sharded 8 ways; edges are
    partitioned by destination row and sorted/padded by (128-row block,
    int16 col bucket) on the host (index prep only - all float math runs
    on device).
  - segment_sum is computed as one-hot matmuls on the PE: for each
    128-edge tile, a [128e x 128r] selection matrix O (O[e,r] = C_e if
    row_e == r) is built with one fused DVE tensor_scalar op, and
    PSUM accumulates O.T @ gathered_features.
  - x[col] source-node features are fetched with dma_gather (SWDGE)
    from DRAM; deg^-1/2 scaling of the *source* side is pre-folded into
    the gathered table (xs = dis * x), the destination side is applied
    per-partition after accumulation.  The layer-2 propagation runs on
    hw2s = dis * (h @ W2.T) (40->64-padded rows) so its gather moves
    64-wide rows instead of 128-wide.
  - 3 NEFF launches with host-side concatenation (pure data movement)
    between them: (1) deg/dis/xs, (2) layer-1 -> hw2s slices,
    (3) layer-2 -> output slices.
"""

import numpy as np

import concourse.bacc as bacc
import concourse.mybir as mybir
import concourse.tile as tile
from concourse.bass_utils import run_bass_kernel_spmd

# ---- problem constants (hardcoded per the task contract) ----
N = 100000
E = 1600000
D = 128            # feature dim
H = 128            # hidden dim
C_OUT = 40         # output classes
C_PAD = 64         # padded output width (dma_gather needs 256B rows)

NCORES = 8
RPC = 12544        # rows per core (8 * 12544 = 100352 >= N)
NPAD = NCORES * RPC
NBLK = RPC // 128  # 98 row blocks per core
NBUCK = 4
BUCK = NPAD // NBUCK  # 25088 rows per int16 col bucket

F32 = mybir.dt.float32
I16 = mybir.dt.int16


def _wrap_idx(flat):
    """dma_gather index layout: idx i -> [i % 16, i // 16], replicated to
    128 partitions."""
    n = flat.shape[0]
    w = flat.reshape(n // 16, 16).T  # [16, n//16]
    return np.tile(w, (8, 1))


def _build_host_layouts(row, col, cv):
    """Sort edges by destination row; build per-core padded layouts.

    Returns per-core dicts of launch-1 edge arrays (row-block tiles) and
    launch-2/3 edge arrays ((block, bucket) gather tiles), plus the tile
    counts T1 (launch-1 tiles per block) and TBB (tiles per block-bucket).
    """
    order = np.argsort(row, kind="stable")
    rs = row[order].astype(np.int64)
    cs = col[order].astype(np.int64)
    ws = cv[order].astype(np.float32)

    core_of = rs // RPC
    per_core = []
    T1 = 1
    TBB = 1
    for c in range(NCORES):
        m = core_of == c
        r_loc = (rs[m] - c * RPC).astype(np.int64)
        cc = cs[m]
        ww = ws[m]
        blk = r_loc // 128
        rl = (r_loc % 128).astype(np.float32)
        bk = cc // BUCK
        crel = (cc % BUCK).astype(np.int16)
        per_core.append((blk, rl, bk, crel, ww))
        cnt_b = np.bincount(blk, minlength=NBLK)
        if cnt_b.size:
            T1 = max(T1, int(np.ceil(cnt_b.max() / 128)))
        cnt_bb = np.bincount(blk * NBUCK + bk, minlength=NBLK * NBUCK)
        if cnt_bb.size:
            TBB = max(TBB, int(np.ceil(cnt_bb.max() / 128)))

    l1_row = np.zeros((NCORES, NBLK * T1, 128), np.float32)
    l1_c = np.zeros((NCORES, NBLK * T1, 128), np.float32)
    l2_row = np.zeros((NCORES, NBLK * NBUCK * TBB, 128), np.float32)
    l2_c = np.zeros((NCORES, NBLK * NBUCK * TBB, 128), np.float32)
    l2_idx = np.zeros((NCORES, NBLK * NBUCK, TBB * 128), np.int16)

    for c in range(NCORES):
        blk, rl, bk, crel, ww = per_core[c]
        # launch-1 layout: edges grouped by block
        o = np.argsort(blk, kind="stable")
        blk1, rl1, ww1 = blk[o], rl[o], ww[o]
        starts = np.searchsorted(blk1, np.arange(NBLK))
        ends = np.searchsorted(blk1, np.arange(NBLK), side="right")
        for b in range(NBLK):
            s, e = starts[b], ends[b]
            n = e - s
            if n == 0:
                continue
            dst = l1_row[c, b * T1:(b + 1) * T1].reshape(-1)
            dst[:n] = rl1[s:e]
            dst2 = l1_c[c, b * T1:(b + 1) * T1].reshape(-1)
            dst2[:n] = ww1[s:e]
        # launch-2/3 layout: edges grouped by (block, bucket)
        key = blk * NBUCK + bk
        o = np.argsort(key, kind="stable")
        key2, rl2, crel2, ww2 = key[o], rl[o], crel[o], ww[o]
        starts = np.searchsorted(key2, np.arange(NBLK * NBUCK))
        ends = np.searchsorted(key2, np.arange(NBLK * NBUCK), side="right")
        for q in range(NBLK * NBUCK):
            s, e = starts[q], ends[q]
            n = e - s
            if n == 0:
                continue
            dst = l2_row[c, q * TBB:(q + 1) * TBB].reshape(-1)
            dst[:n] = rl2[s:e]
            dst2 = l2_c[c, q * TBB:(q + 1) * TBB].reshape(-1)
            dst2[:n] = ww2[s:e]
            l2_idx[c, q, :n] = crel2[s:e]

    # SBUF layouts: [128, ntiles] with tile t in column t
    l1_row = np.ascontiguousarray(l1_row.transpose(0, 2, 1))
    l1_c = np.ascontiguousarray(l1_c.transpose(0, 2, 1))
    l2_row = np.ascontiguousarray(l2_row.transpose(0, 2, 1))
    l2_c = np.ascontiguousarray(l2_c.transpose(0, 2, 1))
    # gather idx: wrapped layout per (block,bucket): [128, TBB*8] each
    l2_idx_w = np.zeros((NCORES, 128, NBLK * NBUCK * TBB * 8), np.int16)
    for c in range(NCORES):
        for q in range(NBLK * NBUCK):
            l2_idx_w[c, :, q * TBB * 8:(q + 1) * TBB * 8] = _wrap_idx(l2_idx[c, q])
    return l1_row, l1_c, l2_row, l2_c, l2_idx_w, T1, TBB


IOTA128 = np.tile(np.arange(128, dtype=np.float32), (128, 1))
IDENT128 = np.eye(128, dtype=np.float32)


def _build_launch1(T1):
    """deg (one-hot matmuls) -> dis -> xs, all row-local per core."""
    nc = bacc.Bacc("TRN2", target_bir_lowering=False)
    x_sl = nc.dram_tensor("x_sl", [RPC, D], F32, kind="ExternalInput")
    rowt = nc.dram_tensor("rowt", [128, NBLK * T1], F32, kind="ExternalInput")
    ct = nc.dram_tensor("ct", [128, NBLK * T1], F32, kind="ExternalInput")
    iota = nc.dram_tensor("iota", [128, 128], F32, kind="ExternalInput")
    xs_sl = nc.dram_tensor("xs_sl", [RPC, D], F32, kind="ExternalOutput")
    dis_sl = nc.dram_tensor("dis_sl", [128, NBLK], F32, kind="ExternalOutput")

    with tile.TileContext(nc) as tc:
        with tc.tile_pool(name="const", bufs=1) as cpool, \
             tc.tile_pool(name="work", bufs=3) as wpool, \
             tc.tile_pool(name="small", bufs=4) as spool, \
             tc.tile_pool(name="psum", bufs=2, space="PSUM") as ppool:
            rl = cpool.tile([128, NBLK * T1], F32)
            cw = cpool.tile([128, NBLK * T1], F32)
            io = cpool.tile([128, 128], F32)
            dis_all = cpool.tile([128, NBLK], F32)
            nc.sync.dma_start(out=rl[:], in_=rowt[:, :])
            nc.sync.dma_start(out=cw[:], in_=ct[:, :])
            nc.sync.dma_start(out=io[:], in_=iota[:, :])
            for b in range(NBLK):
                deg_ps = ppool.tile([128, 1], F32, tag="deg")
                for t in range(T1):
                    k = b * T1 + t
                    oh = wpool.tile([128, 128], F32, tag="oh")
                    nc.vector.tensor_scalar(
                        out=oh[:], in0=io[:],
                        scalar1=rl[:, k:k + 1], scalar2=None,
                        op0=mybir.AluOpType.is_equal,
                    )
                    nc.tensor.matmul(
                        out=deg_ps[:], lhsT=oh[:], rhs=cw[:, k:k + 1],
                        start=(t == 0), stop=(t == T1 - 1),
                    )
                degs = spool.tile([128, 1], F32, tag="degs")
                z = spool.tile([128, 1], F32, tag="z")
                sq = spool.tile([128, 1], F32, tag="sq")
                rec = spool.tile([128, 1], F32, tag="rec")
                nc.vector.tensor_copy(out=degs[:], in_=deg_ps[:])
                nc.vector.tensor_scalar(
                    out=z[:], in0=degs[:], scalar1=0.0, scalar2=None,
                    op0=mybir.AluOpType.is_le)
                nc.vector.tensor_tensor(
                    out=degs[:], in0=degs[:], in1=z[:],
                    op=mybir.AluOpType.add)
                nc.scalar.sqrt(out=sq[:], in_=degs[:])
                nc.vector.reciprocal(out=rec[:], in_=sq[:])
                nc.vector.tensor_scalar(
                    out=z[:], in0=z[:], scalar1=-1.0, scalar2=1.0,
                    op0=mybir.AluOpType.mult, op1=mybir.AluOpType.add)
                nc.vector.tensor_tensor(
                    out=dis_all[:, b:b + 1], in0=rec[:], in1=z[:],
                    op=mybir.AluOpType.mult)
                xt = wpool.tile([128, D], F32, tag="xt")
                nc.sync.dma_start(out=xt[:], in_=x_sl[b * 128:(b + 1) * 128, :])
                xst = wpool.tile([128, D], F32, tag="xst")
                nc.vector.tensor_scalar(
                    out=xst[:], in0=xt[:], scalar1=dis_all[:, b:b + 1],
                    scalar2=None, op0=mybir.AluOpType.mult)
                nc.sync.dma_start(out=xs_sl[b * 128:(b + 1) * 128, :], in_=xst[:])
            nc.sync.dma_start(out=dis_sl[:, :], in_=dis_all[:])
    nc.compile()
    return nc


def _build_spmm_launch(TBB, layer):
    """layer 1: gather xs (128 wide) -> spmm -> linear1+relu -> linear2
       -> hw2s slice.  layer 2: gather hw2s (64 wide) -> spmm -> +b2
       -> out slice."""
    W_IN = D if layer == 1 else C_PAD
    nc = bacc.Bacc("TRN2", target_bir_lowering=False)
    tab = nc.dram_tensor("tab", [NPAD, W_IN], F32, kind="ExternalInput")
    rowt = nc.dram_tensor("rowt", [128, NBLK * NBUCK * TBB], F32,
                          kind="ExternalInput")
    ct = nc.dram_tensor("ct", [128, NBLK * NBUCK * TBB], F32,
                        kind="ExternalInput")
    idxt = nc.dram_tensor("idxt", [128, NBLK * NBUCK * TBB * 8], I16,
                          kind="ExternalInput")
    iota = nc.dram_tensor("iota", [128, 128], F32, kind="ExternalInput")
    dis_sl = nc.dram_tensor("dis_sl", [128, NBLK], F32, kind="ExternalInput")
    if layer == 1:
        ident = nc.dram_tensor("ident", [128, 128], F32, kind="ExternalInput")
        w1t = nc.dram_tensor("w1t", [D, H], F32, kind="ExternalInput")
        b1 = nc.dram_tensor("b1", [H, 1], F32, kind="ExternalInput")
        w2t = nc.dram_tensor("w2t", [H, C_PAD], F32, kind="ExternalInput")
        out_sl = nc.dram_tensor("out_sl", [RPC, C_PAD], F32,
                                kind="ExternalOutput")
    else:
        b2bc = nc.dram_tensor("b2bc", [128, C_PAD], F32, kind="ExternalInput")
        out_sl = nc.dram_tensor("out_sl", [RPC, C_PAD], F32,
                                kind="ExternalOutput")

    with tile.TileContext(nc) as tc:
        with tc.tile_pool(name="const", bufs=1) as cpool, \
             tc.tile_pool(name="gat", bufs=3) as gpool, \
             tc.tile_pool(name="oh", bufs=4) as opool, \
             tc.tile_pool(name="tailA", bufs=2) as tpool, \
             tc.tile_pool(name="psum", bufs=2, space="PSUM") as ppool, \
             tc.tile_pool(name="psum2", bufs=2, space="PSUM") as ppool2:
            rl = cpool.tile([128, NBLK * NBUCK * TBB], F32)
            cw = cpool.tile([128, NBLK * NBUCK * TBB], F32)
            idxs = cpool.tile([128, NBLK * NBUCK * TBB * 8], I16)
            io = cpool.tile([128, 128], F32)
            dis = cpool.tile([128, NBLK], F32)
            nc.sync.dma_start(out=rl[:], in_=rowt[:, :])
            nc.sync.dma_start(out=cw[:], in_=ct[:, :])
            nc.sync.dma_start(out=idxs[:], in_=idxt[:, :])
            nc.sync.dma_start(out=io[:], in_=iota[:, :])
            nc.sync.dma_start(out=dis[:], in_=dis_sl[:, :])
            if layer == 1:
                idn = cpool.tile([128, 128], F32)
                w1s = cpool.tile([D, H], F32)
                b1s = cpool.tile([H, 1], F32)
                w2s = cpool.tile([H, C_PAD], F32)
                nc.sync.dma_start(out=idn[:], in_=ident[:, :])
                nc.sync.dma_start(out=w1s[:], in_=w1t[:, :])
                nc.sync.dma_start(out=b1s[:], in_=b1[:, :])
                nc.sync.dma_start(out=w2s[:], in_=w2t[:, :])
            else:
                b2s = cpool.tile([128, C_PAD], F32)
                nc.sync.dma_start(out=b2s[:], in_=b2bc[:, :])

            for b in range(NBLK):
                acc = ppool.tile([128, W_IN], F32, tag="acc")
                for k in range(NBUCK):
                    q = b * NBUCK + k
                    g = gpool.tile([128, TBB * W_IN], F32, tag="g")
                    nc.gpsimd.dma_gather(
                        g[:].rearrange("p (t d) -> p t d", d=W_IN),
                        tab[k * BUCK:(k + 1) * BUCK, :],
                        idxs[:, q * TBB * 8:(q + 1) * TBB * 8],
                        TBB * 128, TBB * 128, W_IN,
                    )
                    for t in range(TBB):
                        kk = q * TBB + t
                        oh = opool.tile([128, 128], F32, tag="oh")
                        nc.vector.tensor_scalar(
                            out=oh[:], in0=io[:],
                            scalar1=rl[:, kk:kk + 1], scalar2=cw[:, kk:kk + 1],
                            op0=mybir.AluOpType.is_equal,
                            op1=mybir.AluOpType.mult,
                        )
                        nc.tensor.matmul(
                            out=acc[:], lhsT=oh[:],
                            rhs=g[:, t * W_IN:(t + 1) * W_IN],
                            start=(k == 0 and t == 0),
                            stop=(k == NBUCK - 1 and t == TBB - 1),
                        )
                if layer == 1:
                    s_sb = tpool.tile([128, D], F32, tag="s_sb")
                    nc.vector.tensor_scalar(
                        out=s_sb[:], in0=acc[:], scalar1=dis[:, b:b + 1],
                        scalar2=None, op0=mybir.AluOpType.mult)
                    st_ps = ppool2.tile([128, 128], F32, tag="st_ps")
                    nc.tensor.transpose(out=st_ps[:], in_=s_sb[:], identity=idn[:])
                    st_sb = tpool.tile([128, 128], F32, tag="st_sb")
                    nc.vector.tensor_copy(out=st_sb[:], in_=st_ps[:])
                    ht_ps = ppool2.tile([H, 128], F32, tag="ht_ps")
                    nc.tensor.matmul(out=ht_ps[:], lhsT=w1s[:], rhs=st_sb[:],
                                     start=True, stop=True)
                    ht_sb = tpool.tile([H, 128], F32, tag="ht_sb")
                    nc.scalar.activation(
                        out=ht_sb[:], in_=ht_ps[:],
                        func=mybir.ActivationFunctionType.Relu,
                        bias=b1s[:, 0:1], scale=1.0)
                    hw2_ps = ppool2.tile([128, C_PAD], F32, tag="hw2_ps")
                    nc.tensor.matmul(out=hw2_ps[:], lhsT=ht_sb[:], rhs=w2s[:],
                                     start=True, stop=True)
                    hw2_sb = tpool.tile([128, C_PAD], F32, tag="hw2_sb")
                    nc.vector.tensor_scalar(
                        out=hw2_sb[:], in0=hw2_ps[:], scalar1=dis[:, b:b + 1],
                        scalar2=None, op0=mybir.AluOpType.mult)
                    nc.sync.dma_start(
                        out=out_sl[b * 128:(b + 1) * 128, :], in_=hw2_sb[:])
                else:
                    o_sb = tpool.tile([128, C_PAD], F32, tag="o_sb")
                    nc.vector.tensor_scalar(
                        out=o_sb[:], in0=acc[:], scalar1=dis[:, b:b + 1],
                        scalar2=None, op0=mybir.AluOpType.mult)
                    nc.vector.tensor_tensor(
                        out=o_sb[:], in0=o_sb[:], in1=b2s[:],
                        op=mybir.AluOpType.add)
                    nc.sync.dma_start(
                        out=out_sl[b * 128:(b + 1) * 128, :], in_=o_sb[:])
    nc.compile()
    return nc


_CACHE = {}


def kernel(x, edge_index, C_values, W1, b1, W2, b2):
    x = np.asarray(x, np.float32)
    row = np.asarray(edge_index[0], np.int64)
    col = np.asarray(edge_index[1], np.int64)
    cv = np.asarray(C_values, np.float32)
    W1 = np.asarray(W1, np.float32)
    b1v = np.asarray(b1, np.float32)
    W2 = np.asarray(W2, np.float32)
    b2v = np.asarray(b2, np.float32)

    l1r, l1c, l2r, l2c, l2i, T1, TBB = _build_host_layouts(row, col, cv)

    key = (T1, TBB)
    if key not in _CACHE:
        _CACHE[key] = (
            _build_launch1(T1),
            _build_spmm_launch(TBB, 1),
            _build_spmm_launch(TBB, 2),
        )
    nc1, nc2, nc3 = _CACHE[key]

    x_pad = np.zeros((NPAD, D), np.float32)
    x_pad[:N] = x

    cores = list(range(NCORES))
    # --- launch 1: deg/dis/xs ---
    in1 = [
        {"x_sl": x_pad[c * RPC:(c + 1) * RPC],
         "rowt": l1r[c], "ct": l1c[c], "iota": IOTA128}
        for c in cores
    ]
    r1 = run_bass_kernel_spmd(nc1, in1, core_ids=cores, trace=False)
    xs_full = np.concatenate([r1.results[c]["xs_sl"] for c in cores], axis=0)
    dis = [r1.results[c]["dis_sl"] for c in cores]

    # --- launch 2: layer-1 spmm + linears -> hw2s ---
    w1t = np.ascontiguousarray(W1.T)                   # [D, H]
    w2t = np.zeros((H, C_PAD), np.float32)
    w2t[:, :C_OUT] = W2.T
    in2 = [
        {"tab": xs_full, "rowt": l2r[c], "ct": l2c[c], "idxt": l2i[c],
         "iota": IOTA128, "dis_sl": dis[c], "ident": IDENT128,
         "w1t": w1t, "b1": b1v.reshape(H, 1), "w2t": w2t}
        for c in cores
    ]
    r2 = run_bass_kernel_spmd(nc2, in2, core_ids=cores, trace=False)
    hw2s_full = np.concatenate([r2.results[c]["out_sl"] for c in cores], axis=0)

    # --- launch 3: layer-2 spmm + b2 ---
    b2bc = np.zeros((128, C_PAD), np.float32)
    b2bc[:, :C_OUT] = b2v
    in3 = [
        {"tab": hw2s_full, "rowt": l2r[c], "ct": l2c[c], "idxt": l2i[c],
         "iota": IOTA128, "dis_sl": dis[c], "b2bc": b2bc}
        for c in cores
    ]
    r3 = run_bass_kernel_spmd(nc3, in3, core_ids=cores, trace=False)
    out = np.concatenate([r3.results[c]["out_sl"] for c in cores], axis=0)
    return np.ascontiguousarray(out[:N, :C_OUT])
